# revision 5
# baseline (speedup 1.0000x reference)
"""Self-cdist (euclidean) kernel for Trainium2, 8 NeuronCores.

Computes d[i, j] = ||x[i] - x[j]||_2 for x of shape [16384, 32], fp32.

Strategy (data-parallel over rows, per the sharding hint):
  - Each of the 8 cores owns a 2048-row stripe of the output [2048, 16384].
  - Augmented-GEMM formulation: one K=34 matmul per output tile yields the
    complete squared distance:
      lhsT[k, m] = -2*x[m, k] (k < 32),  lhsT[32, m] = 1, lhsT[33, m] = ||x_m||^2
      rhs [k, j] =    x[j, k] (k < 32),  rhs [32, j] = ||x_j||^2, rhs[33, j] = 1
      psum[m, j] = -2*x_m.x_j + ||x_j||^2 + ||x_m||^2 = d2[m, j]
    ACT then does a single sqrt pass PSUM -> SBUF; DMA stores 1 MiB tiles.
  - Matmul operands use dtype float32r (fp32 read, FP22 multiply) which
    streams at 1 column/cycle for moving dim >= 256 (4x faster than fp32).
  - K=34 uses only 34 of the 128 PE rows, so operands are duplicated at
    partition 64 and matmul pairs run concurrently in distinct PE row
    groups via tile_position.
  - The kernel is output-bandwidth bound: 128 MiB of fp32 per core.
"""

import sys

if "/opt/trn_rl_repo" not in sys.path:
    sys.path.insert(0, "/opt/trn_rl_repo")

import numpy as np

N = 16384
D = 32
NCORES = 8
RPC = N // NCORES          # rows per core: 2048
KAUG = D + 2               # augmented contraction dim: 34
MT = RPC // 128            # m-tiles per core: 16
CHUNK = 2048               # output column chunk (1 MiB DMA per store)
NQ = N // CHUNK            # column chunks: 8

_CACHE = {}


def _build_bass():
    import concourse.bacc as bacc
    import concourse.mybir as mybir
    import concourse.tile as tile

    f32 = mybir.dt.float32
    f32r = mybir.dt.float32r

    nc = bacc.Bacc("TRN2", target_bir_lowering=False, debug=False,
                   num_devices=NCORES)
    lhsT_d = nc.dram_tensor("lhsT", [KAUG, RPC], f32r, kind="ExternalInput")
    rhs_d = nc.dram_tensor("rhs", [KAUG, N], f32r, kind="ExternalInput")
    out_d = nc.dram_tensor("out", [RPC, N], f32, kind="ExternalOutput")

    with tile.TileContext(nc) as tc:
        with (
            tc.tile_pool(name="const", bufs=1) as cpool,
            tc.tile_pool(name="psum", bufs=2, space="PSUM") as pspool,
            tc.tile_pool(name="outp", bufs=6) as opool,
        ):
            # Duplicate the K=34 operands at partitions 0 and 64 so pairs of
            # matmuls run concurrently in distinct PE row groups
            # (tile_position row packing — K=34 only uses 34 of 128 rows).
            lhsT = cpool.tile([64 + KAUG, RPC], f32r)
            nc.sync.dma_start(lhsT[0:KAUG, :], lhsT_d.ap()[:])
            nc.sync.dma_start(lhsT[64:64 + KAUG, :], lhsT_d.ap()[:])
            rhs = cpool.tile([64 + KAUG, N], f32r)
            # Chunked loads so the first matmuls start before the whole
            # 16K-column operand lands.
            for nq in range(NQ):
                cs = slice(nq * CHUNK, (nq + 1) * CHUNK)
                nc.sync.dma_start(rhs[0:KAUG, cs], rhs_d.ap()[:, cs])
                nc.sync.dma_start(rhs[64:64 + KAUG, cs], rhs_d.ap()[:, cs])

            out_ap = out_d.ap()
            # sqrt(psum) on ACT, straight from PSUM. Diagonal elements may
            # see sqrt(tiny negative) = NaN from fp22 rounding (true d2 is 0
            # there; off-diagonal min d2 is ~5.7, far above rounding noise)
            # — kernel() pins the diagonal to 0 host-side.
            for mt in range(MT):
                ms = slice(mt * 128, (mt + 1) * 128)
                for nq in range(NQ):
                    ps = pspool.tile([128, CHUNK], f32)
                    for i in range(CHUNK // 512):
                        c0 = nq * CHUNK + i * 512
                        rp = 64 * (i % 2)
                        nc.tensor.matmul(
                            ps[:, i * 512:(i + 1) * 512],
                            lhsT[rp:rp + KAUG, ms],
                            rhs[rp:rp + KAUG, c0:c0 + 512],
                            start=True, stop=True,
                            tile_position=(rp, 0),
                        )
                    ot = opool.tile([128, CHUNK], f32)
                    nc.scalar.activation(
                        ot[:], ps[:], mybir.ActivationFunctionType.Sqrt,
                    )
                    nc.sync.dma_start(
                        out_ap[ms, nq * CHUNK:(nq + 1) * CHUNK],
                        ot[:],
                    )

    nc.compile()
    return nc


def _prep_inputs(x: np.ndarray):
    x = np.ascontiguousarray(np.asarray(x, dtype=np.float32))
    assert x.shape == (N, D), x.shape
    sq = (x * x).sum(axis=1, dtype=np.float32).astype(np.float32)
    xt = np.ascontiguousarray(x.T)                       # [32, 16384]
    ones = np.ones((1, N), np.float32)
    rhs = np.concatenate([xt, sq[None, :], ones], axis=0)        # [34, N]
    lhsT_full = np.concatenate([-2.0 * xt, ones, sq[None, :]], axis=0)
    in_maps = []
    for c in range(NCORES):
        s = slice(c * RPC, (c + 1) * RPC)
        in_maps.append({
            "lhsT": np.ascontiguousarray(lhsT_full[:, s]),
            "rhs": rhs,
        })
    return in_maps


def kernel(x: np.ndarray) -> np.ndarray:
    from concourse import bass_utils

    if "nc" not in _CACHE:
        _CACHE["nc"] = _build_bass()
    nc = _CACHE["nc"]

    in_maps = _prep_inputs(x)
    res = bass_utils.run_bass_kernel_spmd(
        nc, in_maps, core_ids=list(range(NCORES)))
    out = np.concatenate(
        [res.results[c]["out"] for c in range(NCORES)], axis=0)
    # The reference returns exactly 0 on the diagonal; the device value
    # there is sqrt(clamped fp22 rounding noise) — pin it.
    np.fill_diagonal(out, 0.0)
    return out


# revision 7
# speedup vs baseline: 1.2644x; 1.2644x over previous
"""Self-cdist (euclidean) kernel for Trainium2, 8 NeuronCores.

Computes d[i, j] = ||x[i] - x[j]||_2 for x of shape [16384, 32], fp32.

Strategy (data-parallel over rows, per the sharding hint):
  - Each of the 8 cores owns a 2048-row stripe of the output [2048, 16384].
  - Augmented-GEMM formulation: one K=34 matmul per output tile yields the
    complete squared distance:
      lhsT[k, m] = -2*x[m, k] (k < 32),  lhsT[32, m] = 1, lhsT[33, m] = ||x_m||^2
      rhs [k, j] =    x[j, k] (k < 32),  rhs [32, j] = ||x_j||^2, rhs[33, j] = 1
      psum[m, j] = -2*x_m.x_j + ||x_j||^2 + ||x_m||^2 = d2[m, j]
    ACT then does a single sqrt pass PSUM -> SBUF; DMA stores 1 MiB tiles.
  - Matmul operands use dtype float32r (fp32 read, FP22 multiply) which
    streams at 1 column/cycle for moving dim >= 256 (4x faster than fp32).
  - K=34 uses only 34 of the 128 PE rows, so operands are duplicated at
    partition 64 and matmul pairs run concurrently in distinct PE row
    groups via tile_position.
  - The kernel is output-bandwidth bound: 128 MiB of fp32 per core.
"""

import sys

if "/opt/trn_rl_repo" not in sys.path:
    sys.path.insert(0, "/opt/trn_rl_repo")

import numpy as np

N = 16384
D = 32
NCORES = 8
RPC = N // NCORES          # rows per core: 2048
KAUG = D + 2               # augmented contraction dim: 34
MT = RPC // 128            # m-tiles per core: 16
CHUNK = 2048               # output column chunk (1 MiB DMA per store)
NQ = N // CHUNK            # column chunks: 8

_CACHE = {}


def _build_bass():
    import concourse.bacc as bacc
    import concourse.mybir as mybir
    import concourse.tile as tile

    f32 = mybir.dt.float32
    f32r = mybir.dt.float32r

    nc = bacc.Bacc("TRN2", target_bir_lowering=False, debug=False,
                   num_devices=NCORES)
    lhsT_d = nc.dram_tensor("lhsT", [KAUG, RPC], f32r, kind="ExternalInput")
    rhs_d = nc.dram_tensor("rhs", [KAUG, N], f32r, kind="ExternalInput")
    out_d = nc.dram_tensor("out", [RPC, N], f32, kind="ExternalOutput")

    with tile.TileContext(nc) as tc:
        with (
            tc.tile_pool(name="const", bufs=1) as cpool,
            tc.tile_pool(name="psum", bufs=2, space="PSUM") as pspool,
            tc.tile_pool(name="outp", bufs=6) as opool,
        ):
            # Duplicate the K=34 operands at partitions 0 and 64 so pairs of
            # matmuls run concurrently in distinct PE row groups
            # (tile_position row packing — K=34 only uses 34 of 128 rows).
            # Input loads ride SWDGE (gpsimd) so they never queue behind the
            # output stores on the HWDGE rings.
            lhsT = cpool.tile([64 + KAUG, RPC], f32r)
            nc.gpsimd.dma_start(lhsT[0:KAUG, :], lhsT_d.ap()[:])
            nc.gpsimd.dma_start(lhsT[64:64 + KAUG, :], lhsT_d.ap()[:])
            rhs = cpool.tile([64 + KAUG, N], f32r)
            # Chunked loads so the first matmuls start before the whole
            # 16K-column operand lands.
            for nq in range(NQ):
                cs = slice(nq * CHUNK, (nq + 1) * CHUNK)
                nc.gpsimd.dma_start(rhs[0:KAUG, cs], rhs_d.ap()[:, cs])
                nc.gpsimd.dma_start(rhs[64:64 + KAUG, cs], rhs_d.ap()[:, cs])

            out_ap = out_d.ap()
            # sqrt(psum) on ACT, straight from PSUM. Diagonal elements may
            # see sqrt(tiny negative) = NaN from fp22 rounding (true d2 is 0
            # there; off-diagonal min d2 is ~5.7, far above rounding noise)
            # — kernel() pins the diagonal to 0 host-side.
            for mt in range(MT):
                ms = slice(mt * 128, (mt + 1) * 128)
                for nq in range(NQ):
                    ps = pspool.tile([128, CHUNK], f32)
                    for i in range(CHUNK // 512):
                        c0 = nq * CHUNK + i * 512
                        rp = 64 * (i % 2)
                        nc.tensor.matmul(
                            ps[:, i * 512:(i + 1) * 512],
                            lhsT[rp:rp + KAUG, ms],
                            rhs[rp:rp + KAUG, c0:c0 + 512],
                            start=True, stop=True,
                            tile_position=(rp, 0),
                        )
                    ot = opool.tile([128, CHUNK], f32)
                    nc.scalar.activation(
                        ot[:], ps[:], mybir.ActivationFunctionType.Sqrt,
                    )
                    # Alternate stores across the two physical HWDGE rings
                    # (SP and ACT) so ring-drain bubbles overlap.
                    store_eng = nc.sync if (mt * NQ + nq) % 2 == 0 else nc.scalar
                    store_eng.dma_start(
                        out_ap[ms, nq * CHUNK:(nq + 1) * CHUNK],
                        ot[:],
                    )

    nc.compile()
    return nc


def _prep_inputs(x: np.ndarray):
    x = np.ascontiguousarray(np.asarray(x, dtype=np.float32))
    assert x.shape == (N, D), x.shape
    sq = (x * x).sum(axis=1, dtype=np.float32).astype(np.float32)
    xt = np.ascontiguousarray(x.T)                       # [32, 16384]
    ones = np.ones((1, N), np.float32)
    rhs = np.concatenate([xt, sq[None, :], ones], axis=0)        # [34, N]
    lhsT_full = np.concatenate([-2.0 * xt, ones, sq[None, :]], axis=0)
    in_maps = []
    for c in range(NCORES):
        s = slice(c * RPC, (c + 1) * RPC)
        in_maps.append({
            "lhsT": np.ascontiguousarray(lhsT_full[:, s]),
            "rhs": rhs,
        })
    return in_maps


def kernel(x: np.ndarray) -> np.ndarray:
    from concourse import bass_utils

    if "nc" not in _CACHE:
        _CACHE["nc"] = _build_bass()
    nc = _CACHE["nc"]

    in_maps = _prep_inputs(x)
    res = bass_utils.run_bass_kernel_spmd(
        nc, in_maps, core_ids=list(range(NCORES)))
    out = np.concatenate(
        [res.results[c]["out"] for c in range(NCORES)], axis=0)
    # The reference returns exactly 0 on the diagonal; the device value
    # there is sqrt(clamped fp22 rounding noise) — pin it.
    np.fill_diagonal(out, 0.0)
    return out
